# revision 18
# baseline (speedup 1.0000x reference)
"""Distributed multi-head causal attention for TRN2, 8 NeuronCores.

Strategy (tensor-parallel over heads + AllToAll re-shard for the output
projection):
  - Each core owns 2 of the 16 heads. It computes Q,K,V projections for its
    heads over the full sequence (both batches), applies RoPE, and computes
    causal softmax(QK^T/sqrt(hd)) @ V for its heads.
  - Everything on-chip is laid out TRANSPOSED: qT/kT are [hd, B*S], scores are
    [k, q], attention output is [hd, q].  This avoids all transposes:
      scoresT = kT_block.T @ qT        (lhsT=kT block, rhs=qT)
      aoT     = v_block.T  @ pT        (lhsT=v natural [k,hd], rhs=pT [k,q])
    Softmax denominator (sum over k = partition axis) comes from a ones-matmul
    (lhsT=ones [128,128]) that also broadcasts the sum across partitions.
    exp() is computed WITHOUT max subtraction (max |score| ~ 6, safe in f32).
  - Per-batch AllToAll swaps head-sharding for sequence-sharding (overlapped
    with the other batch's attention / wo compute): each core ends with all
    16 heads for its 256-position slice of each batch, then computes its
    slice of the wo projection: outT = woT_chunk.T @ attnT (output
    transposed; host transposes back).  Batch 1's AllToAll is split per head
    so the last collective is half-size and fires earlier (shorter tail).
  - Compute dtype: bf16 matmul operands, f32 PSUM accumulation, f32 softmax.

Queue discipline (engine queues are in-order; a dma_start that waits on a
semaphore blocks everything behind it on that queue):
  - startup loads spread over sync/scalar/vector/gpsimd/tensor queues,
    i-chunk-ordered so the first QK matmul starts ~4us in;
  - phase-2 exp runs on scalar; NOTHING else is queued on scalar during
    phase 2 (g_sb loads that wait on A2A completion go on gpsimd);
  - attention-output stores: batch 0 -> gpsimd (before the A2A instructions),
    batch 1 -> vector (issued right after the ao mul, no lag);
  - phase-3 out stores -> scalar (idle then).

Host-side prep casts inputs to bf16 and pre-transposes x/wo; host-side
assembly transposes/concats per-core outputs.  No host arithmetic.
"""
import math

import ml_dtypes
import numpy as np

import concourse.bass as bass
import concourse.mybir as mybir
from concourse import bacc
from concourse.tile import TileContext

F32 = mybir.dt.float32
BF16 = mybir.dt.bfloat16

N_CORES = 8
CORE_IDS = list(range(N_CORES))
B = 2
S = 2048
D = 2048
H = 16
HD = 128  # head dim
HPC = H // N_CORES  # heads per core = 2
BS = B * S  # 4096
NB = S // 512  # 4 q-free-blocks per batch
NK = S // 128  # 16 k-blocks per batch
SCHUNK = S // N_CORES  # 256 positions per core per batch
INV_SQRT_HD = 1.0 / math.sqrt(HD)

# stream_shuffle mask: swap adjacent partitions within each 32-group
PAIR_SWAP = [i ^ 1 for i in range(32)]


def build():
    nc = bacc.Bacc(None, num_devices=N_CORES)

    xt = nc.declare_dram_parameter("xt", [2 * B, 128, 16, 1024], BF16, isOutput=False)
    wqt = nc.declare_dram_parameter("wqt", [128, 16, HPC * HD], BF16, isOutput=False)
    wkt = nc.declare_dram_parameter("wkt", [128, 16, HPC * HD], BF16, isOutput=False)
    wvt = nc.declare_dram_parameter("wvt", [128, 16, HPC * HD], BF16, isOutput=False)
    wot = nc.declare_dram_parameter("wot", [128, 16, D], BF16, isOutput=False)
    cgrid = nc.declare_dram_parameter("cgrid", [HD, S], BF16, isOutput=False)
    sgrid = nc.declare_dram_parameter("sgrid", [HD, S], BF16, isOutput=False)
    masks = nc.declare_dram_parameter("masks", [HD, 2, 1024], BF16, isOutput=False)
    out_ext = nc.declare_dram_parameter("out", [D, B * SCHUNK], F32, isOutput=True)

    # per-(batch, head) A2A bounce buffers: 512KB collectives fire as soon as
    # each head's attention output lands, so no A2A is tail-exposed
    bnc_in = [
        [nc.dram_tensor(f"bounce_in{b}h{h}", [N_CORES, HD, SCHUNK], BF16)
         for h in range(HPC)]
        for b in range(B)
    ]
    bnc_out = [
        [nc.dram_tensor(f"bounce_out{b}h{h}", [N_CORES, HD, SCHUNK], BF16)
         for h in range(HPC)]
        for b in range(B)
    ]

    bar_in = nc.dram_tensor("bar_in", [1], F32)
    bar_out = nc.dram_tensor("bar_out", [N_CORES], F32, addr_space="Shared")

    with TileContext(nc) as tc:
        with (
            tc.tile_pool(name="persist", bufs=1) as persist,
            tc.tile_pool(name="tmp", bufs=4) as tmp,
        ):
            # ---------------- persistent SBUF tensors ----------------
            mask_sb = persist.tile([128, 2, 1024], BF16, tag="mask")
            ones_sb = persist.tile([128, 128], BF16, tag="ones")
            nc.vector.memset(ones_sb, 1.0)

            # qT/kT per head: [hd=128, BS] bf16 (post-RoPE).
            # v per head: [128, BS] bf16, chunk ik at cols [128*ik,128*(ik+1))
            # holding v rows (k) on partitions, hd on free.
            q_sb = [persist.tile([128, BS], BF16, tag=f"q{h}", name=f"q_sb{h}") for h in range(HPC)]
            k_sb = [persist.tile([128, BS], BF16, tag=f"k{h}", name=f"k_sb{h}") for h in range(HPC)]
            v_sb = [persist.tile([128, BS], BF16, tag=f"v{h}", name=f"v_sb{h}") for h in range(HPC)]

            # ---------------- phase 1: QKV projections + RoPE ----------------
            with (
                tc.tile_pool(name="p1w", bufs=1) as p1w,
                tc.tile_pool(name="xt_pool", bufs=2) as xt_pool,
                tc.tile_pool(name="p1psum", bufs=1, space="PSUM") as p1psum,
                tc.tile_pool(name="p1psumv", bufs=2, space="PSUM") as p1psumv,
            ):
                wq_sb = p1w.tile([128, 16, HPC * HD], BF16, tag="wq")
                wk_sb = p1w.tile([128, 16, HPC * HD], BF16, tag="wk")
                wv_sb = p1w.tile([128, 16, HPC * HD], BF16, tag="wv")
                cg_sb = p1w.tile([128, S], BF16, tag="cg")
                sg_sb = p1w.tile([128, S], BF16, tag="sg")
                # Only sync/scalar/gpsimd queues can issue DMAs, each worth
                # ~90GB/s — the phase-1 front is DMA-bound, so ordering is
                # strictly by need-time: gpsimd carries all three weights
                # i-interleaved (wq crawls with the first q-tiles, wk with
                # the k-tiles, wv by the V pass); xt half 0 streams per-i
                # from sync+scalar with the RoPE grids squeezed in early.
                for g in range(4):
                    sl = slice(4 * g, 4 * (g + 1))
                    nc.gpsimd.dma_start(out=wq_sb[:, sl, :], in_=wqt[:, sl, :])
                    nc.gpsimd.dma_start(out=wk_sb[:, sl, :], in_=wkt[:, sl, :])
                for g in range(4):
                    sl = slice(4 * g, 4 * (g + 1))
                    nc.gpsimd.dma_start(out=wv_sb[:, sl, :], in_=wvt[:, sl, :])
                nc.gpsimd.dma_start(out=mask_sb, in_=masks[:, :, :])
                # dummy AllGather: absorbs cross-core NEFF-launch skew early,
                # so the later AllToAlls see aligned peers.  Kept LAST on the
                # gpsimd queue in phase 1: nothing phase-1-critical may queue
                # behind it in case collectives block their engine queue.
                nc.gpsimd.collective_compute(
                    "AllGather",
                    mybir.AluOpType.bypass,
                    replica_groups=[CORE_IDS],
                    ins=[bar_in[:]],
                    outs=[bar_out[:]],
                )

                for half in range(2 * B):  # half-batches of 1024 positions
                    b, hf = divmod(half, 2)
                    coff = b * S + hf * 1024  # column offset in [D, BS]
                    poff = hf * 1024  # position offset within batch (grids)
                    xt_sb = xt_pool.tile([128, 16, 1024], BF16, tag="xt")
                    if half == 0:
                        # 16 per-i chunks (256KB), i-ordered, alternating
                        # sync/scalar; first-half grids slot in after i3 on
                        # scalar (RoPE needs them a few us later)
                        for i in range(16):
                            eng = nc.sync if i % 2 == 0 else nc.scalar
                            eng.dma_start(
                                out=xt_sb[:, i:i + 1, :],
                                in_=xt[half, :, i:i + 1, :],
                            )
                            if i == 3:
                                nc.scalar.dma_start(
                                    out=cg_sb[:, 0:1024], in_=cgrid[:, 0:1024])
                                nc.scalar.dma_start(
                                    out=sg_sb[:, 0:1024], in_=sgrid[:, 0:1024])
                        nc.scalar.dma_start(out=cg_sb[:, 1024:2048], in_=cgrid[:, 1024:2048])
                        nc.scalar.dma_start(out=sg_sb[:, 1024:2048], in_=sgrid[:, 1024:2048])
                    else:
                        for q4 in range(4):
                            eng = nc.sync if q4 % 2 == 0 else nc.scalar
                            eng.dma_start(
                                out=xt_sb[:, q4 * 4:(q4 + 1) * 4, :],
                                in_=xt[half, :, q4 * 4:(q4 + 1) * 4, :],
                            )

                    # Q, K for both heads: psum [hd, 512] accumulated over d_in
                    for j2 in range(2):
                        ps = {}
                        for kind, w in (("q", wq_sb), ("k", wk_sb)):
                            for h in range(HPC):
                                p = p1psum.tile([128, 512], F32, tag=f"qk{kind}{h}")
                                ps[(kind, h)] = p
                                for i in range(16):
                                    nc.tensor.matmul(
                                        p,
                                        w[:, i, h * HD:(h + 1) * HD],
                                        xt_sb[:, i, j2 * 512:(j2 + 1) * 512],
                                        start=(i == 0),
                                        stop=(i == 15),
                                    )
                        # RoPE: out = t*cos + pairswap(t)*sin_signed  (DVE only)
                        gcol = slice(poff + j2 * 512, poff + (j2 + 1) * 512)
                        ocol = slice(coff + j2 * 512, coff + (j2 + 1) * 512)
                        for kind, dst in (("q", q_sb), ("k", k_sb)):
                            for h in range(HPC):
                                p = ps[(kind, h)]
                                m1 = tmp.tile([128, 512], F32, tag="rope_m1")
                                nc.vector.tensor_mul(m1, p, cg_sb[:, gcol])
                                sh = tmp.tile([128, 512], F32, tag="rope_sh")
                                nc.vector.stream_shuffle(sh, p, PAIR_SWAP)
                                nc.vector.tensor_mul(sh, sh, sg_sb[:, gcol])
                                nc.vector.tensor_add(dst[h][:, ocol], m1, sh)

                    # V for both heads: psum [s=128, 2*HD] accumulated over d_in
                    for s2 in range(8):
                        pv = p1psumv.tile([128, HPC * HD], F32, tag="v")
                        for i in range(16):
                            nc.tensor.matmul(
                                pv,
                                xt_sb[:, i, s2 * 128:(s2 + 1) * 128],
                                wv_sb[:, i, :],
                                start=(i == 0),
                                stop=(i == 15),
                            )
                        sc = hf * 8 + s2
                        ccol = slice((b * NK + sc) * 128, (b * NK + sc + 1) * 128)
                        for h in range(HPC):
                            nc.scalar.copy(
                                out=v_sb[h][:, ccol], in_=pv[:, h * HD:(h + 1) * HD]
                            )

            # ---------------- phases 2+3 pools ----------------
            with (
                tc.tile_pool(name="p23", bufs=1) as p23,
                tc.tile_pool(name="ptile", bufs=6) as ptile,
            ):
                wo_sb = p23.tile([128, 16, D], BF16, tag="wo")
                nc.gpsimd.dma_start(out=wo_sb, in_=wot[:, :, :])
                g_sb = [
                    [p23.tile([128, 8, SCHUNK], BF16, tag=f"g{b}h{h}", name=f"g_sb{b}{h}")
                     for h in range(HPC)]
                    for b in range(B)
                ]

                # ---------------- phase 2: attention (batch-major) ----------------
                # Causal block trim: within the diagonal 512-quad, k-block t
                # (t=0..3, k offset 128t) only reaches q >= 128t, so scores/
                # exp/PV run at widths 512/384/256/128 instead of 4x512.
                # Low diagonal pair packs [512|384] = 896 cols; high pair
                # packs [256|128] = 384 cols (its own 1-bank PSUM tag).
                # Softmax denominator: all pair-sums fold on DVE into acc_d,
                # ONE ones-matmul per q-block broadcasts the k-sum.
                # Scores for pair e+1 are emitted before PV of pair e (lag-1
                # pipeline) so the PE is never bare-waiting on exp.
                with tc.tile_pool(name="p2psum", bufs=2, space="PSUM") as p2psum:
                    for b in range(B):
                        for h in range(HPC):
                            for jq in range(NB):
                                po = p2psum.tile([128, 512], F32, tag="pv")
                                pden = p2psum.tile([128, 512], F32, tag="den", bufs=1)
                                nkb = 4 * jq + 4  # causal: k-blocks 0..4jq+3
                                npair = nkb // 2
                                q0 = b * S + jq * 512
                                unit = {"acc": None}

                                def kblk(ik):
                                    return k_sb[h][:, b * S + ik * 128: b * S + (ik + 1) * 128]

                                def vblk(ik):
                                    return v_sb[h][:, (b * NK + ik) * 128:(b * NK + ik + 1) * 128]

                                def emit_scores(e):
                                    lo_diag = e == 2 * jq
                                    hi = e == 2 * jq + 1
                                    if hi:
                                        wtot, parts = 384, ((2, 256, 0), (3, 384, 256))
                                        psc = p2psum.tile([128, 384], F32, tag="schi", bufs=1, name="pschi")
                                    elif lo_diag:
                                        wtot, parts = 896, ((0, 0, 0), (1, 128, 512))
                                        psc = p2psum.tile([128, 1024], F32, tag="sc", name="psc")
                                    else:
                                        wtot, parts = 1024, (None, None)
                                        psc = p2psum.tile([128, 1024], F32, tag="sc", name="psc")
                                    if e < 2 * jq:  # off-diagonal: two full blocks
                                        for u in range(2):
                                            nc.tensor.matmul(
                                                psc[:, u * 512:(u + 1) * 512],
                                                kblk(2 * e + u),
                                                q_sb[h][:, q0:q0 + 512],
                                                start=True, stop=True,
                                            )
                                    else:  # diagonal pair, trimmed widths
                                        for t, qoff, c0 in parts:
                                            wq_ = 512 - qoff
                                            nc.tensor.matmul(
                                                psc[:, c0:c0 + wq_],
                                                kblk(4 * jq + t),
                                                q_sb[h][:, q0 + qoff:q0 + 512],
                                                start=True, stop=True,
                                            )
                                    p_sb = ptile.tile([128, wtot], BF16, tag="p", name="p_sb")
                                    nc.scalar.activation(
                                        out=p_sb,
                                        in_=psc[:, 0:wtot],
                                        func=mybir.ActivationFunctionType.Exp,
                                        scale=INV_SQRT_HD,
                                    )
                                    return e, p_sb

                                def finish(st):
                                    e, p_sb = st
                                    lo_diag = e == 2 * jq
                                    hi = e == 2 * jq + 1
                                    acc = unit["acc"]
                                    if e < 2 * jq:  # off-diagonal
                                        for u in range(2):
                                            ik = 2 * e + u
                                            nc.tensor.matmul(
                                                po, vblk(ik), p_sb[:, u * 512:(u + 1) * 512],
                                                start=(ik == 0), stop=False,
                                                skip_group_check=True,
                                            )
                                        d_sb = tmp.tile([128, 512], BF16, tag="dpair")
                                        nc.vector.tensor_add(
                                            d_sb, p_sb[:, 0:512], p_sb[:, 512:1024]
                                        )
                                        if acc is None:
                                            unit["acc"] = d_sb
                                        else:
                                            d2 = tmp.tile([128, 512], BF16, tag="dacc")
                                            nc.vector.tensor_add(d2, acc, d_sb)
                                            unit["acc"] = d2
                                    elif lo_diag:
                                        nc.vector.tensor_mul(
                                            p_sb, p_sb, mask_sb[:, 0, 0:896]
                                        )
                                        ik = 4 * jq
                                        nc.tensor.matmul(
                                            po, vblk(ik), p_sb[:, 0:512],
                                            start=(ik == 0), stop=False,
                                            skip_group_check=True,
                                        )
                                        nc.tensor.matmul(
                                            po[:, 128:512], vblk(ik + 1), p_sb[:, 512:896],
                                            start=False, stop=False,
                                            skip_group_check=True,
                                        )
                                        if acc is None:  # jq == 0
                                            d2 = tmp.tile([128, 512], BF16, tag="dacc")
                                            nc.vector.tensor_copy(out=d2, in_=p_sb[:, 0:512])
                                        else:
                                            d2 = tmp.tile([128, 512], BF16, tag="dacc")
                                            nc.vector.tensor_add(d2, acc, p_sb[:, 0:512])
                                        nc.vector.tensor_add(
                                            d2[:, 128:512], d2[:, 128:512], p_sb[:, 512:896]
                                        )
                                        unit["acc"] = d2
                                    else:  # high diagonal pair + unit epilogue
                                        nc.vector.tensor_mul(
                                            p_sb, p_sb, mask_sb[:, 1, 0:384]
                                        )
                                        nc.tensor.matmul(
                                            po[:, 256:512], vblk(4 * jq + 2), p_sb[:, 0:256],
                                            start=False, stop=False,
                                            skip_group_check=True,
                                        )
                                        nc.tensor.matmul(
                                            po[:, 384:512], vblk(4 * jq + 3), p_sb[:, 256:384],
                                            start=False, stop=True,
                                            skip_group_check=True,
                                        )
                                        nc.vector.tensor_add(
                                            acc[:, 256:512], acc[:, 256:512], p_sb[:, 0:256]
                                        )
                                        nc.vector.tensor_add(
                                            acc[:, 384:512], acc[:, 384:512], p_sb[:, 256:384]
                                        )
                                        nc.tensor.matmul(
                                            pden, ones_sb, acc,
                                            start=True, stop=True,
                                            skip_group_check=True,
                                        )

                                prev = emit_scores(0)
                                for e in range(1, npair):
                                    cur = emit_scores(e)
                                    finish(prev)
                                    prev = cur
                                finish(prev)
                                recip = tmp.tile([128, 512], F32, tag="recip")
                                nc.vector.reciprocal_approx_fast(out=recip, in_=pden)
                                ao = tmp.tile([128, 512], BF16, tag="ao", bufs=6)
                                nc.vector.tensor_mul(ao, po, recip)
                                # stores: b1h0 on sync (idle in phase 2);
                                # everything else on gpsimd, interleaved with
                                # the A2A instructions in data order
                                st_eng = nc.sync if (b, h) == (1, 0) else nc.gpsimd
                                for u in range(2):
                                    st_eng.dma_start(
                                        out=bnc_in[b][h][2 * jq + u, :, :],
                                        in_=ao[:, u * 256:(u + 1) * 256],
                                    )
                            # fire this head's A2A as soon as its stores land
                            nc.gpsimd.collective_compute(
                                "AllToAll",
                                mybir.AluOpType.bypass,
                                replica_groups=[CORE_IDS],
                                ins=[bnc_in[b][h][:, :, :]],
                                outs=[bnc_out[b][h][:, :, :]],
                            )
                            # gathered-attention loads wait on their A2A; keep
                            # those waits on gpsimd, NOT the scalar exp queue.
                            # b0 loads chase their own A2A (gpsimd is free
                            # until b1h1's stores); b1h0's load waits one A2A
                            # later so it can't delay b1h1's store issue.
                            if b == 0:
                                nc.gpsimd.dma_start(
                                    out=g_sb[0][h],
                                    in_=bnc_out[0][h].rearrange("j p n -> p j n"),
                                )
                            elif h == 1:
                                nc.gpsimd.dma_start(
                                    out=g_sb[1][0],
                                    in_=bnc_out[1][0].rearrange("j p n -> p j n"),
                                )
                    nc.gpsimd.dma_start(
                        out=g_sb[1][1],
                        in_=bnc_out[1][1].rearrange("j p n -> p j n"),
                    )

                # ---------------- phase 3: output projection ----------------
                # per batch: accumulate even global heads (h=0 chunks) then
                # odd (h=1), so each m-block only gates on the A2As in order
                with tc.tile_pool(name="p3psum", bufs=4, space="PSUM") as p3psum:
                    for b in range(B):
                        for m in range(16):
                            pw = p3psum.tile([128, SCHUNK], F32, tag="wo")
                            for h in range(HPC):
                                for j in range(8):
                                    nc.tensor.matmul(
                                        pw,
                                        wo_sb[:, 2 * j + h, m * 128:(m + 1) * 128],
                                        g_sb[b][h][:, j, :],
                                        start=(h == 0 and j == 0),
                                        stop=(h == 1 and j == 7),
                                    )
                            o_sb = tmp.tile([128, SCHUNK], F32, tag="o")
                            nc.vector.tensor_copy(out=o_sb, in_=pw)
                            nc.scalar.dma_start(
                                out=out_ext[m * 128:(m + 1) * 128, b * SCHUNK:(b + 1) * SCHUNK],
                                in_=o_sb,
                            )

    nc.compile()
    return nc


def prep_inputs(x, freqs_cos, freqs_sin, wq, wk, wv, wo):
    """Host-side shard prep. Returns in_maps (list of 8 dicts)."""
    bf = ml_dtypes.bfloat16
    x = np.asarray(x, dtype=np.float32)
    xtf = x.reshape(BS, D).T.astype(bf)  # [D, BS]
    # partition-major pre-chunk: [half, p, chunk, n] -> fully sequential DMAs
    xt = np.ascontiguousarray(xtf.reshape(16, 128, 2 * B, 1024).transpose(2, 1, 0, 3))
    wot = np.ascontiguousarray(np.asarray(wo, np.float32).T.astype(bf).reshape(16, 128, D).transpose(1, 0, 2))
    cos = np.asarray(freqs_cos, np.float32)
    sin = np.asarray(freqs_sin, np.float32)
    cg = np.empty((HD, S), np.float32)
    sg = np.empty((HD, S), np.float32)
    cg[0::2] = cos.T
    cg[1::2] = cos.T
    sg[0::2] = -sin.T
    sg[1::2] = sin.T
    cg = cg.astype(bf)
    sg = sg.astype(bf)
    # trimmed diagonal masks: block t computed only over q-cols >= 128t, so
    # every sub-block mask is the same lower-left triangle kp <= c at widths
    # 512/384 (low pair, packed [0:512|512:896]) and 256/128 (high pair,
    # packed [0:256|256:384])
    kp = np.arange(HD)[:, None]
    tri = (kp <= np.arange(512)[None, :]).astype(np.float32)
    mk = np.zeros((2, HD, 1024), np.float32)
    mk[0][:, 0:512] = tri
    mk[0][:, 512:896] = tri[:, 0:384]
    mk[1][:, 0:256] = tri[:, 0:256]
    mk[1][:, 256:384] = tri[:, 0:128]
    mk = np.ascontiguousarray(mk.astype(bf).transpose(1, 0, 2))

    in_maps = []
    for c in range(N_CORES):
        rows = slice(c * HPC * HD, (c + 1) * HPC * HD)
        in_maps.append({
            "xt": xt,
            "wqt": np.ascontiguousarray(np.asarray(wq, np.float32)[rows, :].T.astype(bf).reshape(16, 128, HPC * HD).transpose(1, 0, 2)),
            "wkt": np.ascontiguousarray(np.asarray(wk, np.float32)[rows, :].T.astype(bf).reshape(16, 128, HPC * HD).transpose(1, 0, 2)),
            "wvt": np.ascontiguousarray(np.asarray(wv, np.float32)[rows, :].T.astype(bf).reshape(16, 128, HPC * HD).transpose(1, 0, 2)),
            "wot": wot,
            "cgrid": cg,
            "sgrid": sg,
            "masks": mk,
        })
    return in_maps


def assemble(results):
    out = np.empty((B, S, D), np.float32)
    for c in range(N_CORES):
        r = results[c]["out"]  # [D, B*SCHUNK]
        for b in range(B):
            out[b, c * SCHUNK:(c + 1) * SCHUNK, :] = (
                r[:, b * SCHUNK:(b + 1) * SCHUNK].T
            )
    return out


_NC_CACHE = []


def kernel(**inputs):
    """Full-input distributed attention on 8 TRN2 NeuronCores.

    Takes the unsharded inputs (x, freqs_cos, freqs_sin, wq, wk, wv, wo) as
    numpy float32 arrays, runs the SPMD bass kernel on cores 0-7, and
    returns the full [B, S, D] float32 output.
    """
    from concourse.bass_utils import run_bass_kernel_spmd

    if not _NC_CACHE:
        _NC_CACHE.append(build())
    nc = _NC_CACHE[0]
    in_maps = prep_inputs(
        x=inputs["x"],
        freqs_cos=inputs["freqs_cos"],
        freqs_sin=inputs["freqs_sin"],
        wq=inputs["wq"],
        wk=inputs["wk"],
        wv=inputs["wv"],
        wo=inputs["wo"],
    )
    res = run_bass_kernel_spmd(nc, in_maps, CORE_IDS, trace=False)
    return assemble(res.results)


# revision 24
# speedup vs baseline: 1.0062x; 1.0062x over previous
"""Distributed multi-head causal attention for TRN2, 8 NeuronCores.

Strategy (tensor-parallel over heads + AllToAll re-shard for the output
projection):
  - Each core owns 2 of the 16 heads. It computes Q,K,V projections for its
    heads over the full sequence (both batches), applies RoPE, and computes
    causal softmax(QK^T/sqrt(hd)) @ V for its heads.
  - Everything on-chip is laid out TRANSPOSED: qT/kT are [hd, B*S], scores are
    [k, q], attention output is [hd, q].  This avoids all transposes:
      scoresT = kT_block.T @ qT        (lhsT=kT block, rhs=qT)
      aoT     = v_block.T  @ pT        (lhsT=v natural [k,hd], rhs=pT [k,q])
    Softmax denominator (sum over k = partition axis) comes from a ones-matmul
    (lhsT=ones [128,128]) that also broadcasts the sum across partitions.
    exp() is computed WITHOUT max subtraction (max |score| ~ 6, safe in f32).
  - Per-batch AllToAll swaps head-sharding for sequence-sharding (overlapped
    with the other batch's attention / wo compute): each core ends with all
    16 heads for its 256-position slice of each batch, then computes its
    slice of the wo projection: outT = woT_chunk.T @ attnT (output
    transposed; host transposes back).  Batch 1's AllToAll is split per head
    so the last collective is half-size and fires earlier (shorter tail).
  - Compute dtype: bf16 matmul operands, f32 PSUM accumulation, f32 softmax.

Queue discipline (engine queues are in-order; a dma_start that waits on a
semaphore blocks everything behind it on that queue):
  - startup loads spread over sync/scalar/vector/gpsimd/tensor queues,
    i-chunk-ordered so the first QK matmul starts ~4us in;
  - phase-2 exp runs on scalar; NOTHING else is queued on scalar during
    phase 2 (g_sb loads that wait on A2A completion go on gpsimd);
  - attention-output stores: batch 0 -> gpsimd (before the A2A instructions),
    batch 1 -> vector (issued right after the ao mul, no lag);
  - phase-3 out stores -> scalar (idle then).

Host-side prep casts inputs to bf16 and pre-transposes x/wo; host-side
assembly transposes/concats per-core outputs.  No host arithmetic.
"""
import math

import ml_dtypes
import numpy as np

import concourse.bass as bass
import concourse.mybir as mybir
from concourse import bacc
from concourse.tile import TileContext

F32 = mybir.dt.float32
BF16 = mybir.dt.bfloat16

N_CORES = 8
CORE_IDS = list(range(N_CORES))
B = 2
S = 2048
D = 2048
H = 16
HD = 128  # head dim
HPC = H // N_CORES  # heads per core = 2
BS = B * S  # 4096
NB = S // 512  # 4 q-free-blocks per batch
NK = S // 128  # 16 k-blocks per batch
SCHUNK = S // N_CORES  # 256 positions per core per batch
INV_SQRT_HD = 1.0 / math.sqrt(HD)

# stream_shuffle mask: swap adjacent partitions within each 32-group
PAIR_SWAP = [i ^ 1 for i in range(32)]


def build():
    nc = bacc.Bacc(None, num_devices=N_CORES)

    xt = nc.declare_dram_parameter("xt", [2 * B, 128, 16, 1024], BF16, isOutput=False)
    wqt = nc.declare_dram_parameter("wqt", [128, 16, HPC * HD], BF16, isOutput=False)
    wkt = nc.declare_dram_parameter("wkt", [128, 16, HPC * HD], BF16, isOutput=False)
    wvt = nc.declare_dram_parameter("wvt", [128, 16, HPC * HD], BF16, isOutput=False)
    wot = nc.declare_dram_parameter("wot", [128, 16, D], BF16, isOutput=False)
    cgrid = nc.declare_dram_parameter("cgrid", [HD, S], BF16, isOutput=False)
    sgrid = nc.declare_dram_parameter("sgrid", [HD, S], BF16, isOutput=False)
    masks = nc.declare_dram_parameter("masks", [HD, 2, 1024], BF16, isOutput=False)
    out_ext = nc.declare_dram_parameter("out", [D, B * SCHUNK], F32, isOutput=True)

    # per-(batch, head) A2A bounce buffers: 512KB collectives fire as soon as
    # each head's attention output lands, so no A2A is tail-exposed
    bnc_in = [
        [nc.dram_tensor(f"bounce_in{b}h{h}", [N_CORES, HD, SCHUNK], BF16)
         for h in range(HPC)]
        for b in range(B)
    ]
    bnc_out = [
        [nc.dram_tensor(f"bounce_out{b}h{h}", [N_CORES, HD, SCHUNK], BF16)
         for h in range(HPC)]
        for b in range(B)
    ]

    bar_in = nc.dram_tensor("bar_in", [1], F32)
    bar_out = nc.dram_tensor("bar_out", [N_CORES], F32, addr_space="Shared")

    with TileContext(nc) as tc:
        with (
            tc.tile_pool(name="persist", bufs=1) as persist,
            tc.tile_pool(name="tmp", bufs=4) as tmp,
        ):
            # ---------------- persistent SBUF tensors ----------------
            mask_sb = persist.tile([128, 2, 1024], BF16, tag="mask")
            ones_sb = persist.tile([128, 128], BF16, tag="ones")
            nc.vector.memset(ones_sb, 1.0)

            # qT/kT per head: [hd=128, BS] bf16 (post-RoPE).
            # v per head: [128, BS] bf16, chunk ik at cols [128*ik,128*(ik+1))
            # holding v rows (k) on partitions, hd on free.
            q_sb = [persist.tile([128, BS], BF16, tag=f"q{h}", name=f"q_sb{h}") for h in range(HPC)]
            k_sb = [persist.tile([128, BS], BF16, tag=f"k{h}", name=f"k_sb{h}") for h in range(HPC)]
            v_sb = [persist.tile([128, BS], BF16, tag=f"v{h}", name=f"v_sb{h}") for h in range(HPC)]

            # ---------------- phase 1: QKV projections + RoPE ----------------
            # The front is DMA-bound (~90GB/s per queue, 3 queues): QK
            # matmuls are emitted i-MAJOR (all 4 psum tiles advance together
            # per xt chunk) so the accumulation crawl tracks DMA arrival
            # instead of exposing it tile-by-tile, and each half's V pass is
            # DEFERRED one half so wv and the V work sit outside the front.
            with (
                tc.tile_pool(name="p1w", bufs=1) as p1w,
                tc.tile_pool(name="xt_pool", bufs=2) as xt_pool,
                tc.tile_pool(name="p1psum", bufs=1, space="PSUM") as p1psum,
                tc.tile_pool(name="p1psumv", bufs=2, space="PSUM") as p1psumv,
            ):
                wq_sb = p1w.tile([128, 16, HPC * HD], BF16, tag="wq")
                wk_sb = p1w.tile([128, 16, HPC * HD], BF16, tag="wk")
                wv_sb = p1w.tile([128, 16, HPC * HD], BF16, tag="wv")
                cg_sb = p1w.tile([128, S], BF16, tag="cg")
                sg_sb = p1w.tile([128, S], BF16, tag="sg")
                # Only sync/scalar/gpsimd queues can issue DMAs, each worth
                # ~90GB/s — the phase-1 front is DMA-bound, so ordering is
                # strictly by need-time: gpsimd carries all three weights
                # i-interleaved (wq crawls with the first q-tiles, wk with
                # the k-tiles, wv by the V pass); xt half 0 streams per-i
                # from sync+scalar with the RoPE grids squeezed in early.
                for g in range(4):
                    sl = slice(4 * g, 4 * (g + 1))
                    nc.gpsimd.dma_start(out=wq_sb[:, sl, :], in_=wqt[:, sl, :])
                    nc.gpsimd.dma_start(out=wk_sb[:, sl, :], in_=wkt[:, sl, :])
                for g in range(4):
                    sl = slice(4 * g, 4 * (g + 1))
                    nc.gpsimd.dma_start(out=wv_sb[:, sl, :], in_=wvt[:, sl, :])
                nc.gpsimd.dma_start(out=mask_sb, in_=masks[:, :, :])
                # dummy AllGather: absorbs cross-core NEFF-launch skew early,
                # so the later AllToAlls see aligned peers.  Kept LAST on the
                # gpsimd queue in phase 1: nothing phase-1-critical may queue
                # behind it in case collectives block their engine queue.
                nc.gpsimd.collective_compute(
                    "AllGather",
                    mybir.AluOpType.bypass,
                    replica_groups=[CORE_IDS],
                    ins=[bar_in[:]],
                    outs=[bar_out[:]],
                )

                xt_tiles = {}

                def load_xt(half):
                    xt_sb = xt_pool.tile([128, 16, 1024], BF16, tag="xt")
                    xt_tiles[half] = xt_sb
                    # 2-i chunks (512KB), i-ordered, alternating sync/scalar;
                    # first-batch grids slot in early on scalar
                    for g8 in range(8):
                        eng = nc.sync if g8 % 2 == 0 else nc.scalar
                        eng.dma_start(
                            out=xt_sb[:, g8 * 2:(g8 + 1) * 2, :],
                            in_=xt[half, :, g8 * 2:(g8 + 1) * 2, :],
                        )
                        if half == 0 and g8 == 1:
                            nc.scalar.dma_start(
                                out=cg_sb[:, 0:1024], in_=cgrid[:, 0:1024])
                            nc.scalar.dma_start(
                                out=sg_sb[:, 0:1024], in_=sgrid[:, 0:1024])
                    if half == 1:
                        nc.scalar.dma_start(out=cg_sb[:, 1024:2048], in_=cgrid[:, 1024:2048])
                        nc.scalar.dma_start(out=sg_sb[:, 1024:2048], in_=sgrid[:, 1024:2048])

                def rope(p, h, dst, j2, poff, coff):
                    gcol = slice(poff + j2 * 512, poff + (j2 + 1) * 512)
                    ocol = slice(coff + j2 * 512, coff + (j2 + 1) * 512)
                    m1 = tmp.tile([128, 512], F32, tag="rope_m1")
                    nc.vector.tensor_mul(m1, p, cg_sb[:, gcol])
                    sh = tmp.tile([128, 512], F32, tag="rope_sh")
                    nc.vector.stream_shuffle(sh, p, PAIR_SWAP)
                    nc.vector.tensor_mul(sh, sh, sg_sb[:, gcol])
                    nc.vector.tensor_add(dst[h][:, ocol], m1, sh)

                def qk_half(half):
                    b, hf = divmod(half, 2)
                    coff = b * S + hf * 1024  # column offset in [D, BS]
                    poff = hf * 1024  # position offset within batch (grids)
                    xt_sb = xt_tiles[half]
                    tiles = [(kind, w, h, dst)
                             for kind, w, dst in (("q", wq_sb, q_sb), ("k", wk_sb, k_sb))
                             for h in range(HPC)]
                    # j2=0 rides the DMA front: STAGGERED i-major — the 4
                    # accumulation chains advance together, one chunk apart,
                    # so the crawl tracks chunk arrival and the chains stop
                    # (and RoPE) one-by-one
                    ps = {}
                    for kind, w, h, dst in tiles:
                        ps[(kind, h)] = p1psum.tile(
                            [128, 512], F32, tag=f"qk{kind}{h}", name=f"ps_{kind}{h}")
                    for r in range(16 + len(tiles) - 1):
                        for idx, (kind, w, h, dst) in enumerate(tiles):
                            i = r - idx
                            if 0 <= i < 16:
                                nc.tensor.matmul(
                                    ps[(kind, h)],
                                    w[:, i, h * HD:(h + 1) * HD],
                                    xt_sb[:, i, 0:512],
                                    start=(i == 0),
                                    stop=(i == 15),
                                    skip_group_check=True,
                                )
                        if r >= 15:  # chain (r-15) just stopped -> its RoPE
                            kind, w, h, dst = tiles[r - 15]
                            rope(ps[(kind, h)], h, dst, 0, poff, coff)
                    # j2=1: data is resident -> tile-major, RoPE per tile
                    for kind, w, h, dst in tiles:
                        p = p1psum.tile([128, 512], F32, tag=f"qk{kind}{h}")
                        for i in range(16):
                            nc.tensor.matmul(
                                p,
                                w[:, i, h * HD:(h + 1) * HD],
                                xt_sb[:, i, 512:1024],
                                start=(i == 0),
                                stop=(i == 15),
                            )
                        rope(p, h, dst, 1, poff, coff)

                def v_half(half):
                    b, hf = divmod(half, 2)
                    xt_sb = xt_tiles[half]
                    for s2 in range(8):
                        pv = p1psumv.tile([128, HPC * HD], F32, tag="v")
                        for i in range(16):
                            nc.tensor.matmul(
                                pv,
                                xt_sb[:, i, s2 * 128:(s2 + 1) * 128],
                                wv_sb[:, i, :],
                                start=(i == 0),
                                stop=(i == 15),
                            )
                        sc = hf * 8 + s2
                        ccol = slice((b * NK + sc) * 128, (b * NK + sc + 1) * 128)
                        for h in range(HPC):
                            nc.scalar.copy(
                                out=v_sb[h][:, ccol], in_=pv[:, h * HD:(h + 1) * HD]
                            )

                # V runs after BOTH batch-0 halves' QK: wv and the V work
                # stay out of the DMA-bound front (xt_pool bufs=2: half k+2
                # loads only after V frees half k's tile)
                load_xt(0)
                load_xt(1)
                qk_half(0)
                qk_half(1)
                v_half(0)
                load_xt(2)
                v_half(1)
                load_xt(3)
                qk_half(2)
                v_half(2)
                qk_half(3)
                v_half(3)

            # ---------------- phases 2+3 pools ----------------
            with (
                tc.tile_pool(name="p23", bufs=1) as p23,
                tc.tile_pool(name="ptile", bufs=6) as ptile,
            ):
                wo_sb = p23.tile([128, 16, D], BF16, tag="wo")
                nc.gpsimd.dma_start(out=wo_sb, in_=wot[:, :, :])
                g_sb = [
                    [p23.tile([128, 8, SCHUNK], BF16, tag=f"g{b}h{h}", name=f"g_sb{b}{h}")
                     for h in range(HPC)]
                    for b in range(B)
                ]

                # ---------------- phase 2: attention (batch-major) ----------------
                # Causal block trim: within the diagonal 512-quad, k-block t
                # (t=0..3, k offset 128t) only reaches q >= 128t, so scores/
                # exp/PV run at widths 512/384/256/128 instead of 4x512.
                # Low diagonal pair packs [512|384] = 896 cols; high pair
                # packs [256|128] = 384 cols (its own 1-bank PSUM tag).
                # Softmax denominator: all pair-sums fold on DVE into acc_d,
                # ONE ones-matmul per q-block broadcasts the k-sum.
                # Scores for pair e+1 are emitted before PV of pair e (lag-1
                # pipeline) so the PE is never bare-waiting on exp.
                with tc.tile_pool(name="p2psum", bufs=2, space="PSUM") as p2psum:
                    for b in range(B):
                        for h in range(HPC):
                            for jq in range(NB):
                                po = p2psum.tile([128, 512], F32, tag="pv")
                                pden = p2psum.tile([128, 512], F32, tag="den", bufs=1)
                                nkb = 4 * jq + 4  # causal: k-blocks 0..4jq+3
                                npair = nkb // 2
                                q0 = b * S + jq * 512
                                unit = {"acc": None}

                                def kblk(ik):
                                    return k_sb[h][:, b * S + ik * 128: b * S + (ik + 1) * 128]

                                def vblk(ik):
                                    return v_sb[h][:, (b * NK + ik) * 128:(b * NK + ik + 1) * 128]

                                def emit_scores(e):
                                    lo_diag = e == 2 * jq
                                    hi = e == 2 * jq + 1
                                    if hi:
                                        wtot, parts = 384, ((2, 256, 0), (3, 384, 256))
                                        psc = p2psum.tile([128, 384], F32, tag="schi", bufs=1, name="pschi")
                                    elif lo_diag:
                                        wtot, parts = 896, ((0, 0, 0), (1, 128, 512))
                                        psc = p2psum.tile([128, 1024], F32, tag="sc", name="psc")
                                    else:
                                        wtot, parts = 1024, (None, None)
                                        psc = p2psum.tile([128, 1024], F32, tag="sc", name="psc")
                                    if e < 2 * jq:  # off-diagonal: two full blocks
                                        for u in range(2):
                                            nc.tensor.matmul(
                                                psc[:, u * 512:(u + 1) * 512],
                                                kblk(2 * e + u),
                                                q_sb[h][:, q0:q0 + 512],
                                                start=True, stop=True,
                                            )
                                    else:  # diagonal pair, trimmed widths
                                        for t, qoff, c0 in parts:
                                            wq_ = 512 - qoff
                                            nc.tensor.matmul(
                                                psc[:, c0:c0 + wq_],
                                                kblk(4 * jq + t),
                                                q_sb[h][:, q0 + qoff:q0 + 512],
                                                start=True, stop=True,
                                            )
                                    p_sb = ptile.tile([128, wtot], BF16, tag="p", name="p_sb")
                                    nc.scalar.activation(
                                        out=p_sb,
                                        in_=psc[:, 0:wtot],
                                        func=mybir.ActivationFunctionType.Exp,
                                        scale=INV_SQRT_HD,
                                    )
                                    return e, p_sb

                                def finish(st):
                                    e, p_sb = st
                                    lo_diag = e == 2 * jq
                                    hi = e == 2 * jq + 1
                                    acc = unit["acc"]
                                    if e < 2 * jq:  # off-diagonal
                                        for u in range(2):
                                            ik = 2 * e + u
                                            nc.tensor.matmul(
                                                po, vblk(ik), p_sb[:, u * 512:(u + 1) * 512],
                                                start=(ik == 0), stop=False,
                                                skip_group_check=True,
                                            )
                                        d_sb = tmp.tile([128, 512], BF16, tag="dpair")
                                        nc.vector.tensor_add(
                                            d_sb, p_sb[:, 0:512], p_sb[:, 512:1024]
                                        )
                                        if acc is None:
                                            unit["acc"] = d_sb
                                        else:
                                            d2 = tmp.tile([128, 512], BF16, tag="dacc")
                                            nc.vector.tensor_add(d2, acc, d_sb)
                                            unit["acc"] = d2
                                    elif lo_diag:
                                        nc.vector.tensor_mul(
                                            p_sb, p_sb, mask_sb[:, 0, 0:896]
                                        )
                                        ik = 4 * jq
                                        nc.tensor.matmul(
                                            po, vblk(ik), p_sb[:, 0:512],
                                            start=(ik == 0), stop=False,
                                            skip_group_check=True,
                                        )
                                        nc.tensor.matmul(
                                            po[:, 128:512], vblk(ik + 1), p_sb[:, 512:896],
                                            start=False, stop=False,
                                            skip_group_check=True,
                                        )
                                        if acc is None:  # jq == 0
                                            d2 = tmp.tile([128, 512], BF16, tag="dacc")
                                            nc.vector.tensor_copy(out=d2, in_=p_sb[:, 0:512])
                                        else:
                                            d2 = tmp.tile([128, 512], BF16, tag="dacc")
                                            nc.vector.tensor_add(d2, acc, p_sb[:, 0:512])
                                        nc.vector.tensor_add(
                                            d2[:, 128:512], d2[:, 128:512], p_sb[:, 512:896]
                                        )
                                        unit["acc"] = d2
                                    else:  # high diagonal pair + unit epilogue
                                        nc.vector.tensor_mul(
                                            p_sb, p_sb, mask_sb[:, 1, 0:384]
                                        )
                                        nc.tensor.matmul(
                                            po[:, 256:512], vblk(4 * jq + 2), p_sb[:, 0:256],
                                            start=False, stop=False,
                                            skip_group_check=True,
                                        )
                                        nc.tensor.matmul(
                                            po[:, 384:512], vblk(4 * jq + 3), p_sb[:, 256:384],
                                            start=False, stop=True,
                                            skip_group_check=True,
                                        )
                                        nc.vector.tensor_add(
                                            acc[:, 256:512], acc[:, 256:512], p_sb[:, 0:256]
                                        )
                                        nc.vector.tensor_add(
                                            acc[:, 384:512], acc[:, 384:512], p_sb[:, 256:384]
                                        )
                                        nc.tensor.matmul(
                                            pden, ones_sb, acc,
                                            start=True, stop=True,
                                            skip_group_check=True,
                                        )

                                prev = emit_scores(0)
                                for e in range(1, npair):
                                    cur = emit_scores(e)
                                    finish(prev)
                                    prev = cur
                                finish(prev)
                                recip = tmp.tile([128, 512], F32, tag="recip")
                                nc.vector.reciprocal_approx_fast(out=recip, in_=pden)
                                ao = tmp.tile([128, 512], BF16, tag="ao", bufs=6)
                                nc.vector.tensor_mul(ao, po, recip)
                                # stores: b1h0 on sync (idle in phase 2);
                                # everything else on gpsimd, interleaved with
                                # the A2A instructions in data order
                                st_eng = nc.sync if (b, h) == (1, 0) else nc.gpsimd
                                for u in range(2):
                                    st_eng.dma_start(
                                        out=bnc_in[b][h][2 * jq + u, :, :],
                                        in_=ao[:, u * 256:(u + 1) * 256],
                                    )
                            # fire this head's A2A as soon as its stores land
                            nc.gpsimd.collective_compute(
                                "AllToAll",
                                mybir.AluOpType.bypass,
                                replica_groups=[CORE_IDS],
                                ins=[bnc_in[b][h][:, :, :]],
                                outs=[bnc_out[b][h][:, :, :]],
                            )
                            # gathered-attention loads wait on their A2A; keep
                            # those waits on gpsimd, NOT the scalar exp queue.
                            # b0 loads chase their own A2A (gpsimd is free
                            # until b1h1's stores); b1h0's load waits one A2A
                            # later so it can't delay b1h1's store issue.
                            if b == 0:
                                nc.gpsimd.dma_start(
                                    out=g_sb[0][h],
                                    in_=bnc_out[0][h].rearrange("j p n -> p j n"),
                                )
                            elif h == 1:
                                nc.gpsimd.dma_start(
                                    out=g_sb[1][0],
                                    in_=bnc_out[1][0].rearrange("j p n -> p j n"),
                                )
                    nc.gpsimd.dma_start(
                        out=g_sb[1][1],
                        in_=bnc_out[1][1].rearrange("j p n -> p j n"),
                    )

                # ---------------- phase 3: output projection ----------------
                # per batch: accumulate even global heads (h=0 chunks) then
                # odd (h=1), so each m-block only gates on the A2As in order
                with tc.tile_pool(name="p3psum", bufs=4, space="PSUM") as p3psum:
                    for b in range(B):
                        for m in range(16):
                            pw = p3psum.tile([128, SCHUNK], F32, tag="wo")
                            for h in range(HPC):
                                for j in range(8):
                                    nc.tensor.matmul(
                                        pw,
                                        wo_sb[:, 2 * j + h, m * 128:(m + 1) * 128],
                                        g_sb[b][h][:, j, :],
                                        start=(h == 0 and j == 0),
                                        stop=(h == 1 and j == 7),
                                    )
                            o_sb = tmp.tile([128, SCHUNK], F32, tag="o")
                            nc.vector.tensor_copy(out=o_sb, in_=pw)
                            nc.scalar.dma_start(
                                out=out_ext[m * 128:(m + 1) * 128, b * SCHUNK:(b + 1) * SCHUNK],
                                in_=o_sb,
                            )

    nc.compile()
    return nc


def prep_inputs(x, freqs_cos, freqs_sin, wq, wk, wv, wo):
    """Host-side shard prep. Returns in_maps (list of 8 dicts)."""
    bf = ml_dtypes.bfloat16
    x = np.asarray(x, dtype=np.float32)
    xtf = x.reshape(BS, D).T.astype(bf)  # [D, BS]
    # partition-major pre-chunk: [half, p, chunk, n] -> fully sequential DMAs
    xt = np.ascontiguousarray(xtf.reshape(16, 128, 2 * B, 1024).transpose(2, 1, 0, 3))
    wot = np.ascontiguousarray(np.asarray(wo, np.float32).T.astype(bf).reshape(16, 128, D).transpose(1, 0, 2))
    cos = np.asarray(freqs_cos, np.float32)
    sin = np.asarray(freqs_sin, np.float32)
    cg = np.empty((HD, S), np.float32)
    sg = np.empty((HD, S), np.float32)
    cg[0::2] = cos.T
    cg[1::2] = cos.T
    sg[0::2] = -sin.T
    sg[1::2] = sin.T
    cg = cg.astype(bf)
    sg = sg.astype(bf)
    # trimmed diagonal masks: block t computed only over q-cols >= 128t, so
    # every sub-block mask is the same lower-left triangle kp <= c at widths
    # 512/384 (low pair, packed [0:512|512:896]) and 256/128 (high pair,
    # packed [0:256|256:384])
    kp = np.arange(HD)[:, None]
    tri = (kp <= np.arange(512)[None, :]).astype(np.float32)
    mk = np.zeros((2, HD, 1024), np.float32)
    mk[0][:, 0:512] = tri
    mk[0][:, 512:896] = tri[:, 0:384]
    mk[1][:, 0:256] = tri[:, 0:256]
    mk[1][:, 256:384] = tri[:, 0:128]
    mk = np.ascontiguousarray(mk.astype(bf).transpose(1, 0, 2))

    in_maps = []
    for c in range(N_CORES):
        rows = slice(c * HPC * HD, (c + 1) * HPC * HD)
        in_maps.append({
            "xt": xt,
            "wqt": np.ascontiguousarray(np.asarray(wq, np.float32)[rows, :].T.astype(bf).reshape(16, 128, HPC * HD).transpose(1, 0, 2)),
            "wkt": np.ascontiguousarray(np.asarray(wk, np.float32)[rows, :].T.astype(bf).reshape(16, 128, HPC * HD).transpose(1, 0, 2)),
            "wvt": np.ascontiguousarray(np.asarray(wv, np.float32)[rows, :].T.astype(bf).reshape(16, 128, HPC * HD).transpose(1, 0, 2)),
            "wot": wot,
            "cgrid": cg,
            "sgrid": sg,
            "masks": mk,
        })
    return in_maps


def assemble(results):
    out = np.empty((B, S, D), np.float32)
    for c in range(N_CORES):
        r = results[c]["out"]  # [D, B*SCHUNK]
        for b in range(B):
            out[b, c * SCHUNK:(c + 1) * SCHUNK, :] = (
                r[:, b * SCHUNK:(b + 1) * SCHUNK].T
            )
    return out


_NC_CACHE = []


def kernel(**inputs):
    """Full-input distributed attention on 8 TRN2 NeuronCores.

    Takes the unsharded inputs (x, freqs_cos, freqs_sin, wq, wk, wv, wo) as
    numpy float32 arrays, runs the SPMD bass kernel on cores 0-7, and
    returns the full [B, S, D] float32 output.
    """
    from concourse.bass_utils import run_bass_kernel_spmd

    if not _NC_CACHE:
        _NC_CACHE.append(build())
    nc = _NC_CACHE[0]
    in_maps = prep_inputs(
        x=inputs["x"],
        freqs_cos=inputs["freqs_cos"],
        freqs_sin=inputs["freqs_sin"],
        wq=inputs["wq"],
        wk=inputs["wk"],
        wv=inputs["wv"],
        wo=inputs["wo"],
    )
    res = run_bass_kernel_spmd(nc, in_maps, CORE_IDS, trace=False)
    return assemble(res.results)
